# revision 14
# baseline (speedup 1.0000x reference)
"""GCN layer on 8 trn2 NeuronCores.

out = tanh( (D^-1/2 (adj+I) D^-1/2) @ H @ W.T + b ), N=8192, nin=nout=512.

Identities used:
  1. D^-1/2 A D^-1/2 @ H = (d_m ⊙ A) @ (d_k ⊙ H), d = deg^-0.5 — the row
     scale is folded into the adjacency (host), the col scale into H.
  2. (A @ Hs) @ W.T = A @ (Hs @ W.T) — the small GEMM G = Hs @ W.T
     (4.3 GFLOP) runs on host, so the device does ONE big matmul chain
     per core (A_rowblock @ G, 8.6 GFLOP bf16) plus bias + tanh.
  3. Self loops folded into the adjacency diagonal on host.

Everything the PE consumes is bf16 (1 cycle/row vs 4 for fp32; halves
HBM traffic). PSUM accumulates fp32, output is fp32. L2 rel err ~2e-3.

Per-core layout (partition-major so every DMA has >=16KB contiguous
lines): S[p, kt, m] = d[m_g] * A_full[m_g, kt*128+p] for the core's
1024 output rows; G[p, kt, n] = ((d ⊙ H) @ W.T)[kt*128+p, n] full.

Device: 8 PSUM banks = 8 live accumulators (one per 128-row m-tile).
Stream S in 8 x 2MB strips (sync HWDGE ring) overlapped with the
64-k-tile matmul chain; G prefetched in 4 x 2MB chunks (scalar HWDGE
ring). Bias via rank-1 ones^T @ b matmul into each accumulator, tanh
on readout (scalar engine), 8 x 256KB output DMAs.
"""

import sys

sys.path.insert(0, "/opt/trn_rl_repo")

import numpy as np
import ml_dtypes

from concourse import bass, bacc, tile, mybir
from concourse.bass_utils import run_bass_kernel_spmd

N = 8192
NIN = 512
NOUT = 512
NC = 8
RB = N // NC  # 1024 rows per core
MT = RB // 128  # 8 m-tiles per core
KT = N // 128  # 64 k-tiles
KOCT = 8  # k-tiles per S strip
F32 = mybir.dt.float32
BF16 = mybir.dt.bfloat16
NPBF16 = ml_dtypes.bfloat16

_CACHED_NC = None


def _build():
    nc = bacc.Bacc(None, target_bir_lowering=False)

    S = nc.dram_tensor("S", [128, KT, RB], BF16, kind="ExternalInput")
    G = nc.dram_tensor("G", [128, KT, NOUT], BF16, kind="ExternalInput")
    Bb = nc.dram_tensor("Bb", [1, NOUT], BF16, kind="ExternalInput")
    Out = nc.dram_tensor("out", [RB, NOUT], BF16, kind="ExternalOutput")

    with tile.TileContext(nc) as tc:
        with (
            tc.tile_pool(name="persist", bufs=1) as persist,
            tc.tile_pool(name="strip", bufs=5) as striper,
            tc.tile_pool(name="outs", bufs=2) as outp,
            tc.tile_pool(name="acc", bufs=1, space=bass.MemorySpace.PSUM) as pacc,
        ):
            # G resident: [128, kt, n]; ramped chunks on the scalar HWDGE
            # ring so the PE's first moving operand lands early.
            g_big = persist.tile([128, KT, NOUT], BF16)
            g0 = 0
            for gch in (2, 6, 8, 16, 16, 16):
                nc.scalar.dma_start(
                    g_big[:, g0 : g0 + gch, :], G[:, g0 : g0 + gch, :]
                )
                g0 += gch
            b_t = persist.tile([1, NOUT], BF16)
            nc.gpsimd.dma_start(b_t[:], Bb[:, :])
            ones_t = persist.tile([1, 128], BF16)
            nc.gpsimd.memset(ones_t[:], 1.0)

            accs = [
                pacc.tile([128, NOUT], F32, name=f"acc{m}") for m in range(MT)
            ]

            # S strips on the sync HWDGE ring: ramped sizes so the PE
            # starts ~10us earlier; last strip runs m-outer so readout
            # (bias matmul, tanh, out-DMA) overlaps the PE's final MMs.
            strips = (1, 3, 4, 8, 8, 8, 8, 8, 8, 8)
            assert sum(strips) == KT
            k0 = 0
            for si, nk in enumerate(strips):
                last = si == len(strips) - 1
                strip = striper.tile([128, KOCT, RB], BF16, name="strip")
                nc.sync.dma_start(
                    strip[:, :nk, :], S[:, k0 : k0 + nk, :]
                )
                if not last:
                    for j in range(nk):
                        kt = k0 + j
                        for m in range(MT):
                            nc.tensor.matmul(
                                accs[m][:],
                                strip[:, j, m * 128 : (m + 1) * 128],
                                g_big[:, kt, :],
                                start=(kt == 0),
                                stop=False,
                            )
                else:
                    for m in range(MT):
                        for j in range(nk):
                            kt = k0 + j
                            nc.tensor.matmul(
                                accs[m][:],
                                strip[:, j, m * 128 : (m + 1) * 128],
                                g_big[:, kt, :],
                                start=False,
                                stop=False,
                            )
                        # += ones ⊗ b, closing m's accumulation group
                        nc.tensor.matmul(
                            accs[m][:], ones_t[:], b_t[:], start=False, stop=True
                        )
                        res = outp.tile([128, NOUT], BF16)
                        if m < MT - 1:
                            nc.scalar.activation(
                                res[:],
                                accs[m][:],
                                mybir.ActivationFunctionType.Tanh,
                            )
                            nc.sync.dma_start(
                                Out[m * 128 : (m + 1) * 128, :], res[:]
                            )
                        else:
                            # split the last readout so the final (exposed)
                            # out-DMA is half-sized
                            for hv in range(2):
                                cs = slice(hv * (NOUT // 2), (hv + 1) * (NOUT // 2))
                                nc.scalar.activation(
                                    res[:, cs],
                                    accs[m][:, cs],
                                    mybir.ActivationFunctionType.Tanh,
                                )
                                nc.sync.dma_start(
                                    Out[m * 128 : (m + 1) * 128, cs],
                                    res[:, cs],
                                )
                k0 += nk

    nc.compile()
    return nc


def kernel(H, adj_matrix, W, b):
    global _CACHED_NC
    H = np.asarray(H, dtype=np.float32)
    adj = np.asarray(adj_matrix, dtype=np.float32)
    W = np.asarray(W, dtype=np.float32)
    b = np.asarray(b, dtype=np.float32)

    # Host glue: degrees, d = deg^-0.5, G = (d ⊙ H) @ W.T, and the
    # scaled/bf16/partition-major adjacency row-blocks.
    deg = adj.sum(axis=0, dtype=np.float32) + 1.0  # +1 self loop
    d = deg**-0.5
    d = np.where(np.isinf(d), np.float32(0.0), d).astype(np.float32)
    G32 = (d[:, None] * H) @ W.T
    Gh = np.ascontiguousarray(
        G32.reshape(KT, 128, NOUT).transpose(1, 0, 2).astype(NPBF16)
    )
    Bv = b.astype(NPBF16).reshape(1, NOUT)

    in_maps = []
    diag = np.arange(RB)
    for c in range(NC):
        r0, r1 = c * RB, (c + 1) * RB
        tmp = np.ascontiguousarray(adj[r0:r1, :].T)  # [k, m_local] fp32
        tmp *= d[r0:r1][None, :]  # fold output-row scale
        tmp[r0 + diag, diag] += d[r0:r1]  # self loop: +1 * d_m at k == m_glob
        S_c = np.ascontiguousarray(
            tmp.reshape(KT, 128, RB).transpose(1, 0, 2).astype(NPBF16)
        )
        in_maps.append({"S": S_c, "G": Gh, "Bb": Bv})

    if _CACHED_NC is None:
        _CACHED_NC = _build()
    globals()["_LAST_IN_MAPS"] = in_maps
    res = run_bass_kernel_spmd(_CACHED_NC, in_maps, core_ids=list(range(NC)))
    return np.concatenate(
        [res.results[c]["out"].astype(np.float32) for c in range(NC)], axis=0
    )


# revision 15
# speedup vs baseline: 1.0240x; 1.0240x over previous
"""GCN layer on 8 trn2 NeuronCores.

out = tanh( (D^-1/2 (adj+I) D^-1/2) @ H @ W.T + b ), N=8192, nin=nout=512.

Identities used:
  1. D^-1/2 A D^-1/2 @ H = (d_m ⊙ A) @ (d_k ⊙ H), d = deg^-0.5 — the row
     scale is folded into the adjacency (host), the col scale into H.
  2. (A @ Hs) @ W.T = A @ (Hs @ W.T) — the small GEMM G = Hs @ W.T
     (4.3 GFLOP) runs on host, so the device does ONE big matmul chain
     per core (A_rowblock @ G, 8.6 GFLOP bf16) plus bias + tanh.
  3. Self loops folded into the adjacency diagonal on host.

Everything the PE consumes is bf16 (1 cycle/row vs 4 for fp32; halves
HBM traffic). PSUM accumulates fp32, output is fp32. L2 rel err ~2e-3.

Per-core layout (partition-major so every DMA has >=16KB contiguous
lines): S[p, kt, m] = d[m_g] * A_full[m_g, kt*128+p] for the core's
1024 output rows; G[p, kt, n] = ((d ⊙ H) @ W.T)[kt*128+p, n] full.

Device: 8 PSUM banks = 8 live accumulators (one per 128-row m-tile).
Stream S in 8 x 2MB strips (sync HWDGE ring) overlapped with the
64-k-tile matmul chain; G prefetched in 4 x 2MB chunks (scalar HWDGE
ring). Bias via rank-1 ones^T @ b matmul into each accumulator, tanh
on readout (scalar engine), 8 x 256KB output DMAs.
"""

import sys

sys.path.insert(0, "/opt/trn_rl_repo")

import numpy as np
import ml_dtypes

from concourse import bass, bacc, tile, mybir
from concourse.bass_utils import run_bass_kernel_spmd

N = 8192
NIN = 512
NOUT = 512
NC = 8
RB = N // NC  # 1024 rows per core
MT = RB // 128  # 8 m-tiles per core
KT = N // 128  # 64 k-tiles
KOCT = 8  # k-tiles per S strip
F32 = mybir.dt.float32
BF16 = mybir.dt.bfloat16
NPBF16 = ml_dtypes.bfloat16

_CACHED_NC = None


def _build():
    nc = bacc.Bacc(None, target_bir_lowering=False)

    S = nc.dram_tensor("S", [128, KT, RB], BF16, kind="ExternalInput")
    G = nc.dram_tensor("G", [128, KT, NOUT], BF16, kind="ExternalInput")
    Bb = nc.dram_tensor("Bb", [1, NOUT], BF16, kind="ExternalInput")
    Out = nc.dram_tensor("out", [RB, NOUT], BF16, kind="ExternalOutput")

    with tile.TileContext(nc) as tc:
        with (
            tc.tile_pool(name="persist", bufs=1) as persist,
            tc.tile_pool(name="strip", bufs=5) as striper,
            tc.tile_pool(name="outs", bufs=2) as outp,
            tc.tile_pool(name="acc", bufs=1, space=bass.MemorySpace.PSUM) as pacc,
        ):
            # G resident: [128, kt, n]; ramped chunks on the scalar HWDGE
            # ring so the PE's first moving operand lands early.
            g_big = persist.tile([128, KT, NOUT], BF16)
            g0 = 0
            for gch in (2, 6, 8, 16, 16, 16):
                nc.scalar.dma_start(
                    g_big[:, g0 : g0 + gch, :], G[:, g0 : g0 + gch, :]
                )
                g0 += gch
            b_t = persist.tile([1, NOUT], BF16)
            nc.gpsimd.dma_start(b_t[:], Bb[:, :])
            ones_t = persist.tile([1, 128], BF16)
            nc.gpsimd.memset(ones_t[:], 1.0)

            accs = [
                pacc.tile([128, NOUT], F32, name=f"acc{m}") for m in range(MT)
            ]

            # S strips on the sync HWDGE ring: ramped sizes so the PE
            # starts ~10us earlier; last strip runs m-outer so readout
            # (bias matmul, tanh, out-DMA) overlaps the PE's final MMs.
            strips = (2, 2, 4, 8, 8, 8, 8, 8, 8, 8)
            assert sum(strips) == KT
            k0 = 0
            for si, nk in enumerate(strips):
                last = si == len(strips) - 1
                strip = striper.tile([128, KOCT, RB], BF16, name="strip")
                nc.sync.dma_start(
                    strip[:, :nk, :], S[:, k0 : k0 + nk, :]
                )
                if not last:
                    for j in range(nk):
                        kt = k0 + j
                        for m in range(MT):
                            nc.tensor.matmul(
                                accs[m][:],
                                strip[:, j, m * 128 : (m + 1) * 128],
                                g_big[:, kt, :],
                                start=(kt == 0),
                                stop=False,
                            )
                else:
                    for m in range(MT):
                        for j in range(nk):
                            kt = k0 + j
                            nc.tensor.matmul(
                                accs[m][:],
                                strip[:, j, m * 128 : (m + 1) * 128],
                                g_big[:, kt, :],
                                start=False,
                                stop=False,
                            )
                        # += ones ⊗ b, closing m's accumulation group
                        nc.tensor.matmul(
                            accs[m][:], ones_t[:], b_t[:], start=False, stop=True
                        )
                        res = outp.tile([128, NOUT], BF16)
                        if m < MT - 1:
                            nc.scalar.activation(
                                res[:],
                                accs[m][:],
                                mybir.ActivationFunctionType.Tanh,
                            )
                            nc.sync.dma_start(
                                Out[m * 128 : (m + 1) * 128, :], res[:]
                            )
                        else:
                            # split the last readout so the final (exposed)
                            # out-DMA is half-sized
                            for hv in range(2):
                                cs = slice(hv * (NOUT // 2), (hv + 1) * (NOUT // 2))
                                nc.scalar.activation(
                                    res[:, cs],
                                    accs[m][:, cs],
                                    mybir.ActivationFunctionType.Tanh,
                                )
                                nc.sync.dma_start(
                                    Out[m * 128 : (m + 1) * 128, cs],
                                    res[:, cs],
                                )
                k0 += nk

    nc.compile()
    return nc


def kernel(H, adj_matrix, W, b):
    global _CACHED_NC
    H = np.asarray(H, dtype=np.float32)
    adj = np.asarray(adj_matrix, dtype=np.float32)
    W = np.asarray(W, dtype=np.float32)
    b = np.asarray(b, dtype=np.float32)

    # Host glue: degrees, d = deg^-0.5, G = (d ⊙ H) @ W.T, and the
    # scaled/bf16/partition-major adjacency row-blocks.
    deg = adj.sum(axis=0, dtype=np.float32) + 1.0  # +1 self loop
    d = deg**-0.5
    d = np.where(np.isinf(d), np.float32(0.0), d).astype(np.float32)
    G32 = (d[:, None] * H) @ W.T
    Gh = np.ascontiguousarray(
        G32.reshape(KT, 128, NOUT).transpose(1, 0, 2).astype(NPBF16)
    )
    Bv = b.astype(NPBF16).reshape(1, NOUT)

    in_maps = []
    diag = np.arange(RB)
    for c in range(NC):
        r0, r1 = c * RB, (c + 1) * RB
        tmp = np.ascontiguousarray(adj[r0:r1, :].T)  # [k, m_local] fp32
        tmp *= d[r0:r1][None, :]  # fold output-row scale
        tmp[r0 + diag, diag] += d[r0:r1]  # self loop: +1 * d_m at k == m_glob
        S_c = np.ascontiguousarray(
            tmp.reshape(KT, 128, RB).transpose(1, 0, 2).astype(NPBF16)
        )
        in_maps.append({"S": S_c, "G": Gh, "Bb": Bv})

    if _CACHED_NC is None:
        _CACHED_NC = _build()
    globals()["_LAST_IN_MAPS"] = in_maps
    res = run_bass_kernel_spmd(_CACHED_NC, in_maps, core_ids=list(range(NC)))
    return np.concatenate(
        [res.results[c]["out"].astype(np.float32) for c in range(NC)], axis=0
    )
